# revision 2
# baseline (speedup 1.0000x reference)
"""GRU-D layer kernel for Trainium2, 8 NeuronCores, batch-parallel.

Problem shapes: x [256, 512, 128], h_decay [256, 512], H=256.
Sharding: batch 256 -> 32 per core; GRU weights replicated.

Per-core layout (recurrence tensors transposed: h-chunks on 128
partitions, batch on free dim):
  hT [128, 64] : col j = (chunk c=j//32, batch b=j%32), h-index = 128*c + p.

v2 design (PE-instruction minimized):
  Phase 1 writes projT = W^T @ xT + bias DIRECTLY into the step psum
  banks for a 4-step group (one x-tile), eliminating the per-step fold
  matmuls and the proj SBUF round-trip of v1:
    ZR psum bank [128, 512]: col = 128*q + 32*ts + b,
        q in {z c0, z c1, r c0, r c1}
    H  psum bank [128, 256]: col = 128*c + 32*ts + b
  Bias enters via a selector matmul: out = B6^T @ Esel (K=6).
  Per step only the 12 U-accumulation matmuls run on PE (bf16, FWL),
  ordered r -> z -> h so sigmoid(r) fires as early as possible; phase 1
  for the next group fills the PE bubble during tanh/state-update.
"""

import numpy as np

B, T, D, H = 256, 512, 128, 256
NCORES = 8
BS = B // NCORES  # 32

TRACE = False
LAST_EXEC_NS = None

_NC_CACHE = {}


def _build(T_steps, variant=()):
    vset = set(variant)
    import concourse.bass as bass
    import concourse.mybir as mybir
    from concourse.tile import TileContext

    f32 = mybir.dt.float32
    bf16 = mybir.dt.bfloat16
    SIG = mybir.ActivationFunctionType.Sigmoid
    TANH = mybir.ActivationFunctionType.Tanh
    MUL = mybir.AluOpType.mult
    SUB = mybir.AluOpType.subtract

    NT = T_steps // 4        # x-tiles (4 timesteps per tile, 128 bt rows)
    GS = 16 if T_steps >= 16 else T_steps  # steps per output/dec DMA group
    NG = T_steps // GS

    nc = bass.Bass()
    xT_d = nc.dram_tensor("xT", [NT, 128, 128], bf16, kind="ExternalInput")
    W_d = nc.dram_tensor("W", [128, 768], bf16, kind="ExternalInput")
    b6_d = nc.dram_tensor("B6", [6, 128], bf16, kind="ExternalInput")
    esel_d = nc.dram_tensor("Esel", [6, 768], bf16, kind="ExternalInput")
    uzr_d = nc.dram_tensor("Uzr", [8, 128, 128], bf16, kind="ExternalInput")
    uh_d = nc.dram_tensor("Uh4", [4, 128, 128], bf16, kind="ExternalInput")
    # decb/outG laid out exactly like their SBUF tiles: [group, 128
    # partitions, GS*64] with col = 64*t_local + (chunk*32 + batch).
    decb_d = nc.dram_tensor("decb", [NG, 128, GS * 64], f32,
                            kind="ExternalInput")
    outG_d = nc.dram_tensor("outG", [NG, 128, GS * 64], f32,
                            kind="ExternalOutput")

    with TileContext(nc) as tc:
        with (
            tc.tile_pool(name="res", bufs=1) as res,
            tc.tile_pool(name="x1", bufs=3) as x1,
            tc.tile_pool(name="zr", bufs=2, space="PSUM") as zrp,
            tc.tile_pool(name="hh", bufs=2, space="PSUM") as hhp,
            tc.tile_pool(name="hp", bufs=3) as hpool,
            tc.tile_pool(name="wk", bufs=3) as wk,
            tc.tile_pool(name="dec", bufs=4) as decp,
        ):
            # ---- resident constants ----
            w_sb = res.tile([128, 768], bf16)
            nc.sync.dma_start(out=w_sb, in_=W_d[:])
            b6_sb = res.tile([6, 128], bf16)
            nc.sync.dma_start(out=b6_sb, in_=b6_d[:])
            esel_sb = res.tile([6, 768], bf16)
            nc.sync.dma_start(out=esel_sb, in_=esel_d[:])
            uzr = res.tile([128, 1024], bf16)
            nc.sync.dma_start(
                out=uzr[:].rearrange("p (i m) -> p i m", i=8),
                in_=uzr_d.rearrange("i p m -> p i m"),
            )
            uh = res.tile([128, 512], bf16)
            nc.sync.dma_start(
                out=uh[:].rearrange("p (i m) -> p i m", i=4),
                in_=uh_d.rearrange("i p m -> p i m"),
            )

            def load_x(k):
                xt = x1.tile([128, 128], bf16, tag="xt")
                nc.sync.dma_start(out=xt, in_=xT_d[k])
                return xt

            # ---- phase 1: bias + x-projection into a 4-step psum group ----
            def p1_bias(zr_t, hp_t2):
                if "no_p1" in vset:
                    return
                nc.tensor.matmul(zr_t[:, 0:512], b6_sb[:], esel_sb[:, 0:512],
                                 start=True, stop=False, skip_group_check=True)
                nc.tensor.matmul(hp_t2[:, 0:256], b6_sb[:],
                                 esel_sb[:, 512:768],
                                 start=True, stop=False, skip_group_check=True)

            def p1_w(zr_t, hp_t2, xt, part):
                if "no_p1" in vset:
                    return
                if part == 0:
                    for q in range(4):
                        nc.tensor.matmul(
                            zr_t[:, 128 * q:128 * q + 128],
                            w_sb[:, 128 * q:128 * q + 128], xt[:],
                            start=False, stop=False, skip_group_check=True)
                else:
                    for c in range(2):
                        nc.tensor.matmul(
                            hp_t2[:, 128 * c:128 * c + 128],
                            w_sb[:, 512 + 128 * c:640 + 128 * c], xt[:],
                            start=False, stop=False, skip_group_check=True)

            def load_decg(g):
                dt_ = decp.tile([128, GS * 64], f32, tag="db")
                if "no_decb" not in vset:
                    nc.sync.dma_start(out=dt_, in_=decb_d[g])
                else:
                    nc.any.memzero(dt_)
                return dt_

            # strided [chunk, b] views of a psum group for one step
            def zr_view(zr_t, ts, gate):  # gate 0=z (q 0,1), 1=r (q 2,3)
                v = zr_t[:, 256 * gate:256 * gate + 256].rearrange(
                    "p (c t b) -> p t c b", c=2, t=4, b=32)
                return v[:, ts:ts + 1, :, :]

            def hh_view(hp_t2, ts):
                v = hp_t2[:, 0:256].rearrange(
                    "p (c t b) -> p t c b", c=2, t=4, b=32)
                return v[:, ts:ts + 1, :, :]

            # ---- prologue ----
            h0 = res.tile([128, 64], f32)
            nc.any.memzero(h0)
            h_prev = h0[:]

            xt_cur = load_x(0)
            zr_t = zrp.tile([128, 512], f32, tag="zr")
            hp_t2 = hhp.tile([128, 256], f32, tag="hps")
            p1_bias(zr_t, hp_t2)
            p1_w(zr_t, hp_t2, xt_cur, 0)
            p1_w(zr_t, hp_t2, xt_cur, 1)

            decg = load_decg(0)
            decg_nxt = None
            hgrp = None
            xt_nxt = None
            zr_nxt = None
            hp_nxt = None

            # ---- main loop ----
            for t in range(T_steps):
                k, ts = t // 4, t % 4
                g, tg = t // GS, t % GS

                if ts == 0 and k + 1 < NT:
                    xt_nxt = load_x(k + 1)

                if tg == 0:
                    if g + 1 < NG:
                        decg_nxt = load_decg(g + 1)
                    hgrp = hpool.tile([128, GS * 64], f32, tag="hh")
                db = decg[:, 64 * tg:64 * tg + 64]

                hdec = wk.tile([128, 64], bf16, tag="hdec")
                if "no_dve" not in vset:
                    nc.vector.tensor_tensor(out=hdec, in0=h_prev,
                                            in1=db, op=MUL)

                # r then z gate accumulation (r first: it gates h_prop)
                if "no_umm" not in vset:
                    for mc in range(2):
                        for kc in range(2):
                            i0 = ((2 + mc) * 2 + kc) * 128
                            nc.tensor.matmul(
                                zr_t[:, 256 + 128 * mc + 32 * ts:
                                     256 + 128 * mc + 32 * ts + 32],
                                uzr[:, i0:i0 + 128],
                                hdec[:, 32 * kc:32 * kc + 32],
                                start=False, stop=(kc == 1),
                                skip_group_check=True)
                    for mc in range(2):
                        for kc in range(2):
                            i0 = (mc * 2 + kc) * 128
                            nc.tensor.matmul(
                                zr_t[:, 128 * mc + 32 * ts:
                                     128 * mc + 32 * ts + 32],
                                uzr[:, i0:i0 + 128],
                                hdec[:, 32 * kc:32 * kc + 32],
                                start=False, stop=(kc == 1),
                                skip_group_check=True)

                r_bf = wk.tile([128, 64], bf16, tag="rbf")
                z_s = wk.tile([128, 64], f32, tag="zs")
                if "no_act" not in vset:
                    nc.scalar.activation(out=r_bf, in_=zr_view(zr_t, ts, 1),
                                         func=SIG)
                    nc.scalar.activation(out=z_s, in_=zr_view(zr_t, ts, 0),
                                         func=SIG)

                rh = wk.tile([128, 64], bf16, tag="rh")
                if "no_dve" not in vset:
                    nc.vector.tensor_tensor(out=rh, in0=r_bf[:],
                                            in1=hdec[:], op=MUL)

                if "no_umm" not in vset:
                    for mc in range(2):
                        for kc in range(2):
                            i0 = (mc * 2 + kc) * 128
                            nc.tensor.matmul(
                                hp_t2[:, 128 * mc + 32 * ts:
                                      128 * mc + 32 * ts + 32],
                                uh[:, i0:i0 + 128],
                                rh[:, 32 * kc:32 * kc + 32],
                                start=False, stop=(kc == 1),
                                skip_group_check=True)

                # next group's phase 1 fills the PE bubble (tanh + state
                # update) behind the h-matmuls of ts 1 and 2
                if k + 1 < NT:
                    if ts == 1:
                        zr_nxt = zrp.tile([128, 512], f32, tag="zr")
                        hp_nxt = hhp.tile([128, 256], f32, tag="hps")
                        p1_bias(zr_nxt, hp_nxt)
                        p1_w(zr_nxt, hp_nxt, xt_nxt, 0)
                    elif ts == 2:
                        p1_w(zr_nxt, hp_nxt, xt_nxt, 1)

                hp_t = wk.tile([128, 64], f32, tag="hpt")
                if "no_act" not in vset:
                    nc.scalar.activation(out=hp_t, in_=hh_view(hp_t2, ts),
                                         func=TANH)

                # h_new = z*h_prop - (z-1)*dec*h_prev; a1/a2 run during tanh
                a1 = wk.tile([128, 64], f32, tag="a1")
                a2 = wk.tile([128, 64], f32, tag="a2")
                b2 = wk.tile([128, 64], f32, tag="b2")
                h_new = hgrp[:, 64 * tg:64 * tg + 64]
                if "no_dve" not in vset:
                    nc.vector.scalar_tensor_tensor(out=a1, in0=z_s[:],
                                                   scalar=1.0, in1=db,
                                                   op0=SUB, op1=MUL)
                    nc.vector.tensor_tensor(out=a2, in0=a1[:], in1=h_prev,
                                            op=MUL)
                    nc.vector.tensor_tensor(out=b2, in0=z_s[:],
                                            in1=hp_t[:], op=MUL)
                    nc.vector.tensor_tensor(out=h_new, in0=b2[:],
                                            in1=a2[:], op=SUB)

                if ts == 3 and k + 1 < NT:
                    xt_cur = xt_nxt
                    zr_t = zr_nxt
                    hp_t2 = hp_nxt
                if tg == GS - 1:
                    if "no_out" not in vset:
                        nc.sync.dma_start(out=outG_d[g], in_=hgrp[:])
                    decg = decg_nxt
                h_prev = h_new

    _split_matmul_waits(nc, mybir)
    return nc


def _split_matmul_waits(nc, mybir):
    """Walrus allows at most one sync wait per engine instruction. Move the
    excess onto same-engine NoOps inserted just before (avoids
    InstEventSemaphore, which is subject to the cayman event-accel
    deadlock)."""
    for func in nc.m.functions:
        for blk in func.blocks:
            new_insts = []
            for inst in blk.instructions:
                si = inst.sync_info
                if si is not None and len(si.on_wait) > 1:
                    extra = list(si.on_wait[:-1])
                    keep = [si.on_wait[-1]]
                    for w in extra:
                        nop = mybir.InstNoOp(
                            name=nc.get_next_instruction_name(),
                            sync_info=mybir.SyncInfo(on_wait=[w], on_update=[]),
                            engine=inst.engine,
                            bass_nofuse=True,
                        )
                        nc.register_instruction(nop)
                        new_insts.append(nop)
                    si.on_wait = keep
                new_insts.append(inst)
            blk.instructions[:] = new_insts


def _get_nc(T_steps=T, variant=()):
    key = (T_steps, tuple(variant))
    if key not in _NC_CACHE:
        _NC_CACHE[key] = _build(T_steps, variant)
    return _NC_CACHE[key]


def _prep_shared(Wr, Wz, Wh, Ur, Uz, Uh, br, bz, bh):
    import ml_dtypes
    bf = ml_dtypes.bfloat16
    Wz, Wr, Wh = (np.asarray(a, np.float32) for a in (Wz, Wr, Wh))
    Uz, Ur, Uh = (np.asarray(a, np.float32) for a in (Uz, Ur, Uh))
    W = np.ascontiguousarray(
        np.concatenate([Wz, Wr, Wh], axis=1)).astype(bf)
    B6 = np.stack([np.asarray(bz)[0:128], np.asarray(bz)[128:256],
                   np.asarray(br)[0:128], np.asarray(br)[128:256],
                   np.asarray(bh)[0:128], np.asarray(bh)[128:256]]
                  ).astype(bf)
    # Esel[q, col]: ZR cols 0:512 ordered (q, ts, b) -> block q of 128;
    # H cols 512:768 ordered (c, ts, b) -> row 4+c.
    Esel = np.zeros((6, 768), np.float32)
    for q in range(4):
        Esel[q, 128 * q:128 * q + 128] = 1.0
    for c in range(2):
        Esel[4 + c, 512 + 128 * c:640 + 128 * c] = 1.0
    Esel = Esel.astype(bf)
    Uzr = np.empty((8, 128, 128), bf)
    for m in range(4):
        g = Uz if m < 2 else Ur
        mc = m % 2
        for kc in range(2):
            Uzr[m * 2 + kc] = g[128 * kc:128 * kc + 128,
                                128 * mc:128 * mc + 128].astype(bf)
    Uh4 = np.empty((4, 128, 128), bf)
    for mc in range(2):
        for kc in range(2):
            Uh4[mc * 2 + kc] = Uh[128 * kc:128 * kc + 128,
                                  128 * mc:128 * mc + 128].astype(bf)
    return dict(W=W, B6=B6, Esel=Esel, Uzr=Uzr, Uh4=Uh4)


def _prep_core(xs, ds, T_steps):
    # xs [32, T, 128] -> xT [T//4, 128d, 128bt]; col = 32*t_sub + b
    import ml_dtypes
    bf = ml_dtypes.bfloat16
    xs = np.asarray(xs, np.float32)
    ds = np.asarray(ds, np.float32)
    nt = T_steps // 4
    xr = xs.reshape(BS, nt, 4, 128).transpose(1, 3, 2, 0).reshape(nt, 128, 128)
    gs = 16 if T_steps >= 16 else T_steps
    ng = T_steps // gs
    # decb[g, p, 64*t' + 32*c + b] = ds[b, g*gs + t']  (independent of p, c)
    dT = ds.T.reshape(ng, gs, BS)                       # [g, t', b]
    db = np.concatenate([dT, dT], axis=2).reshape(ng, 1, gs * 64)
    decb = np.ascontiguousarray(
        np.broadcast_to(db, (ng, 128, gs * 64)).astype(np.float32))
    return dict(xT=np.ascontiguousarray(xr.astype(bf)), decb=decb)


def _run_spmd(nc, in_maps, n_timed=0):
    """Replicates bass2jax.run_bass_via_pjrt's multi-core path, optionally
    re-executing the compiled body with device-resident inputs to measure
    per-run wall time."""
    import time
    import jax
    import jax.numpy as jnp
    from jax.sharding import Mesh, PartitionSpec
    from jax.experimental.shard_map import shard_map
    import concourse.mybir as mybir
    from concourse import bass2jax
    from concourse.bass2jax import _bass_exec_p, partition_id_tensor

    bass2jax.install_neuronx_cc_hook()
    if not nc.is_finalized():
        nc.finalize()

    partition_name = (nc.partition_id_tensor.name
                      if nc.partition_id_tensor else None)
    in_names, out_names, out_avals, zero_outs = [], [], [], []
    for alloc in nc.m.functions[0].allocations:
        if not isinstance(alloc, mybir.MemoryLocationSet):
            continue
        name = alloc.memorylocations[0].name
        if alloc.kind == "ExternalInput":
            if name != partition_name:
                in_names.append(name)
        elif alloc.kind == "ExternalOutput":
            aval = jax.core.ShapedArray(
                tuple(alloc.tensor_shape), mybir.dt.np(alloc.dtype))
            out_names.append(name)
            out_avals.append(aval)
            zero_outs.append(np.zeros(aval.shape, aval.dtype))

    n_params = len(in_names)
    all_names = list(in_names) + list(out_names)
    if partition_name is not None:
        all_names.append(partition_name)

    def _body(*args):
        operands = list(args)
        if partition_name is not None:
            operands.append(partition_id_tensor())
        return tuple(_bass_exec_p.bind(
            *operands,
            out_avals=tuple(out_avals),
            in_names=tuple(all_names),
            out_names=tuple(out_names),
            lowering_input_output_aliases=(),
            sim_require_finite=True,
            sim_require_nnan=True,
            nc=nc,
        ))

    devices = jax.devices()[:NCORES]
    mesh = Mesh(np.asarray(devices), ("core",))
    nio = n_params + len(out_names)
    sharded = jax.jit(shard_map(
        _body, mesh=mesh,
        in_specs=(PartitionSpec("core"),) * nio,
        out_specs=(PartitionSpec("core"),) * len(out_names),
        check_rep=False), keep_unused=True)

    concat_in = [np.concatenate([np.asarray(m[name]) for m in in_maps], axis=0)
                 for name in in_names]
    concat_zeros = [np.zeros((NCORES * z.shape[0], *z.shape[1:]), z.dtype)
                    for z in zero_outs]
    args = concat_in + concat_zeros

    out_arrs = sharded(*args)
    jax.block_until_ready(out_arrs)

    times = []
    if n_timed:
        # Axon dispatch costs ~100ms per blocked round-trip, so time N
        # queued (unblocked) executions and difference totals: the device
        # runs them back-to-back.
        sharding = jax.sharding.NamedSharding(mesh, PartitionSpec("core"))
        dev_args = [jax.device_put(a, sharding) for a in args]
        jax.block_until_ready(dev_args)

        def _timed(n):
            t0 = time.perf_counter()
            o = None
            for _ in range(n):
                o = sharded(*dev_args)
            jax.block_until_ready(o)
            return time.perf_counter() - t0

        _timed(1)  # warm
        t1 = min(_timed(1) for _ in range(6))
        tn = min(_timed(1 + n_timed) for _ in range(3))
        times = [(tn - t1) / n_timed]

    results = [
        {name: np.asarray(out_arrs[i]).reshape(NCORES, *out_avals[i].shape)[c]
         for i, name in enumerate(out_names)}
        for c in range(NCORES)
    ]
    return results, times


def _make_in_maps(x, h_decay, Wr, Wz, Wh, Ur, Uz, Uh, br, bz, bh, T_steps=T):
    shared = _prep_shared(Wr, Wz, Wh, Ur, Uz, Uh, br, bz, bh)
    x = np.asarray(x, np.float32)
    h_decay = np.asarray(h_decay, np.float32)
    in_maps = []
    for c in range(NCORES):
        m = dict(shared)
        m.update(_prep_core(x[c * BS:(c + 1) * BS],
                            h_decay[c * BS:(c + 1) * BS], T_steps))
        in_maps.append(m)
    return in_maps


def kernel(x, h_decay, Wr, Wz, Wh, Ur, Uz, Uh, br, bz, bh):
    global LAST_EXEC_NS
    nc = _get_nc(T)
    in_maps = _make_in_maps(x, h_decay, Wr, Wz, Wh, Ur, Uz, Uh, br, bz, bh)
    n_timed = 5 if TRACE else 0
    results, times = _run_spmd(nc, in_maps, n_timed=n_timed)
    if times:
        LAST_EXEC_NS = int(min(times) * 1e9)

    out = np.empty((B, T, H), np.float32)
    for c in range(NCORES):
        out[c * BS:(c + 1) * BS] = _unshard_out(results[c]["outG"], T)
    return out


def _unshard_out(oG, T_steps):
    gs = 16 if T_steps >= 16 else T_steps
    ng = T_steps // gs
    # oG [g, p, 64t'+32c+b] -> [b, t, h=128c+p]
    o = oG.reshape(ng, 128, gs, 2, BS)          # [g, p, t', c, b]
    return o.transpose(4, 0, 2, 3, 1).reshape(BS, T_steps, H)
